# revision 10
# baseline (speedup 1.0000x reference)
"""Trainium2 Bass kernel for nn_CommonFeatureExtractor (v2, b-major mid).

Data-parallel over 8 NeuronCores: batch dim (4096) sharded into 8 x 512,
weights replicated.

v2 layout strategy: layer-1 GEMMs run in the transposed layout (h.T
[dh, b], fed by x.T), but layer-2 GEMMs flip to "b-major": lhsT =
h.T[:, b-tile] so fps comes out as [b(128-part) x h(free)] per b-tile of
128 samples.  In b-major every per-sample scalar (pair dot d, norm ss,
softmax weights wq/fpw, fallback mf) is a [P,1] per-partition column, so:
  - d and ss fall out of fused accum_out on the DVE/ACT ops that compute
    the pair products / squares anyway (no ones-matmuls, no row DMAs);
  - the softmax chains are tiny [128,10] ops; reciprocal is [128,1];
  - no partition-broadcast matmuls at all;
  - the masked aggregation is a chain of fused affine_then_add ops
    (acc = msum_p * wq_p[P,1] + acc), one DVE op per pair.
Pair mask-sum slabs msum_p = (fps_i*fps_j>0)*(fps_i+fps_j) are built
during phase A (DVE idle there).  Phase D transposes common/wsum back to
h-major via PE-transposes and runs enhance+fuse GEMMs as before; the
fus-lo half accumulates early (during C) to keep the PE warm.
"""

import numpy as np

import concourse.bass as bass
import concourse.mybir as mybir
import concourse.tile as tile
from concourse import bacc
from concourse.bass_utils import run_bass_kernel_spmd

F32 = mybir.dt.float32
FP16 = mybir.dt.float16
ALU = mybir.AluOpType
AF = mybir.ActivationFunctionType

N_CORES = 8
B = 4096
BC = B // N_CORES  # 512 samples per core
H = 512
P = 128
NBT = BC // P  # 4 b-tiles per core

AP_D, MA_D, MB_D, MC_D, PH_D = 2048, 167, 2048, 2048, 27
# encoders: (name, din, padded K tiles, hidden dh)
ENCS = [
    ("ap", AP_D, 16, 512),
    ("ma", MA_D, 2, 256),
    ("mb", MB_D, 16, 512),
    ("mc", MC_D, 16, 512),
    ("ph", PH_D, 1, 128),
]
XT_K = sum(e[2] for e in ENCS)  # 51 padded k-tiles of x
XT_OFF = np.cumsum([0] + [e[2] for e in ENCS])[:-1]  # [0,16,18,34,50]

_I = [0, 0, 0, 0, 1, 1, 1, 2, 2, 3]
_J = [1, 2, 3, 4, 2, 3, 4, 3, 4, 4]
PAIR_IDX = {(_I[p], _J[p]): p for p in range(10)}
# compute order: small encoders first so pair work overlaps phase A
ORDER = ["ma", "ph", "ap", "mb", "mc"]
ENC_BY_NAME = {e[0]: (i, e) for i, e in enumerate(ENCS)}
# pair-completion order given ORDER (for the aggregation chains: pairs whose
# msum slabs finish last go last)
ORDER_PAIRS = [6, 0, 3, 4, 8, 1, 2, 5, 7, 9]

MID = FP16


def build_bass():
    nc = bacc.Bacc("TRN2", target_bir_lowering=False, debug=False)

    # ---------------- DRAM I/O ----------------
    xt = nc.dram_tensor("xt", [XT_K * P, BC], FP16, kind="ExternalInput")
    w1 = {}
    w2 = {}
    b1 = {}
    b2r = {}
    wgp = {}
    for name, _, K, dh in ENCS:
        w1[name] = nc.dram_tensor(f"w1_{name}", [K * P, dh], FP16, kind="ExternalInput")
        w2[name] = nc.dram_tensor(f"w2_{name}", [dh, H], FP16, kind="ExternalInput")
        b1[name] = nc.dram_tensor(f"b1_{name}", [P, dh // P], F32, kind="ExternalInput")
        b2r[name] = nc.dram_tensor(f"b2r_{name}", [1, H], FP16, kind="ExternalInput")
        wgp[name] = nc.dram_tensor(f"wgp_{name}", [dh, 5], FP16, kind="ExternalInput")
    z0 = nc.dram_tensor("z0", [5, 1], F32, kind="ExternalInput")
    id128 = nc.dram_tensor("id128", [P, P], FP16, kind="ExternalInput")
    enh_w = nc.dram_tensor("enh_w", [H, H], FP16, kind="ExternalInput")
    enh_b = nc.dram_tensor("enh_b", [P, 4], F32, kind="ExternalInput")
    fus_w = nc.dram_tensor("fus_w", [2 * H, H], FP16, kind="ExternalInput")
    fus_b = nc.dram_tensor("fus_b", [P, 4], F32, kind="ExternalInput")
    out = nc.dram_tensor("out", [H, BC], F32, kind="ExternalOutput")

    with tile.TileContext(nc) as tc:
        kernel_body(
            tc, xt, w1, w2, b1, b2r, wgp, z0, id128, enh_w, enh_b, fus_w, fus_b, out
        )
    nc.compile()
    return nc


def kernel_body(tc, xt, w1, w2, b1, b2r, wgp, z0, id128, enh_w, enh_b, fus_w,
                fus_b, out):
    nc = tc.nc
    import contextlib

    ctx = contextlib.ExitStack()
    with ctx:
        # -------- pools --------
        persist = ctx.enter_context(tc.tile_pool(name="persist", bufs=1))
        scr_pool = ctx.enter_context(tc.tile_pool(name="scr", bufs=4))
        msk_pool = ctx.enter_context(tc.tile_pool(name="msk", bufs=2))
        cb_pool = ctx.enter_context(tc.tile_pool(name="cb", bufs=2))
        xt_pool = ctx.enter_context(tc.tile_pool(name="xtp", bufs=3))
        w_pool = ctx.enter_context(tc.tile_pool(name="wp", bufs=3))
        h_pool = ctx.enter_context(tc.tile_pool(name="hp", bufs=2))
        sq_pool = ctx.enter_context(tc.tile_pool(name="sqp", bufs=2))
        gate_pool = ctx.enter_context(tc.tile_pool(name="gatep", bufs=2))
        psum_l1 = ctx.enter_context(tc.tile_pool(name="psl1", bufs=4, space="PSUM"))
        psum_l2 = ctx.enter_context(tc.tile_pool(name="psl2", bufs=3, space="PSUM"))
        psum_z = ctx.enter_context(tc.tile_pool(name="psz", bufs=1, space="PSUM"))

        # -------- persistent tiles --------
        fps_bt = persist.tile([P, NBT, 5, H], MID)      # b-major fps
        msum = persist.tile([P, 10, NBT, H], MID)       # masked pair sums
        s_b = persist.tile([P, NBT, H], MID)            # sum_i fps_i
        stats = persist.tile([P, NBT, 16], F32)         # cols 0-9 d, 10-14 ss
        pl_t = persist.tile([P, NBT, 10], MID)
        lss_t = persist.tile([P, NBT, 5], MID)
        invnn_t = persist.tile([P, NBT, 10], MID)
        sims_t = persist.tile([P, NBT, 10], MID)
        es_t = persist.tile([P, NBT, 10], MID)
        e_t = persist.tile([P, NBT, 10], MID)
        den_t = persist.tile([P, NBT], F32)
        den1_t = persist.tile([P, NBT], F32)
        rden_t = persist.tile([P, NBT], F32)
        r05_t = persist.tile([P, NBT], F32)
        wq_t = persist.tile([P, NBT, 10], F32)
        mf_t = persist.tile([P, NBT], F32)
        ez_bt = persist.tile([P, NBT, 5], MID)
        sez_t = persist.tile([P, NBT], F32)
        rsez_t = persist.tile([P, NBT], F32)
        fpw_t = persist.tile([P, NBT, 5], F32)
        common_b = persist.tile([P, NBT, H], MID)
        wsum_b = persist.tile([P, NBT, H], MID)
        common_h = persist.tile([P, 4, BC], MID)
        wsum_h = persist.tile([P, 4, BC], MID)
        enh_sb = persist.tile([P, 4, BC], MID)
        id_sb = persist.tile([P, P], FP16)
        ones_row = persist.tile([1, P], FP16)
        warmz = persist.tile([1, BC], MID)
        b1_sb = {}
        b2r_sb = {}
        wgp_sb = {}
        for name, _, K, dh in ENCS:
            b1_sb[name] = persist.tile([P, dh // P], F32, name=f"b1sb_{name}")
            b2r_sb[name] = persist.tile([1, H], FP16, name=f"b2r_{name}")
            wgp_sb[name] = persist.tile([P, dh // P, 5], FP16, name=f"wgp_{name}")
        z0_sb = persist.tile([5, 1], F32)
        enhb_sb = persist.tile([P, 4], F32)
        fusb_sb = persist.tile([P, 4], F32)
        ew_t = persist.tile([P, 4, 512], FP16, name="ew_t")
        fw_lo = persist.tile([P, 4, 512], FP16, name="fw_lo")
        fw_hi = persist.tile([P, 4, 512], FP16, name="fw_hi")

        nc.vector.memset(ones_row, 1.0)
        nc.vector.memset(warmz, 0.0)
        # PE warmup during DMA preamble (HAM unthrottle); K=1 matmuls on
        # memset-only operands so no DMA dependency
        for _wu in range(10):
            wu_ps = psum_l2.tile([P, H], F32, tag="l2ps", name=f"warm{_wu}")
            nc.tensor.matmul(wu_ps, ones_row, warmz, start=True, stop=True)
        nc.gpsimd.dma_start(id_sb, id128.ap())
        for name, _, K, dh in ENCS:
            nc.scalar.dma_start(b1_sb[name], b1[name].ap())
            nc.scalar.dma_start(b2r_sb[name], b2r[name].ap())
            nc.gpsimd.dma_start(
                wgp_sb[name], wgp[name].ap().rearrange("(ko p) m -> p ko m", p=P)
            )
        nc.gpsimd.dma_start(z0_sb, z0.ap())
        nc.scalar.dma_start(enhb_sb, enh_b.ap())
        nc.scalar.dma_start(fusb_sb, fus_b.ap())

        xt_view = xt.ap().rearrange("(ko p) n -> p ko n", p=P)

        # ================= Phase A: encoders + pair prep ==================
        z_ps = psum_z.tile([5, BC], F32, tag="zps", name="zgate")
        Z_MM_TOTAL = sum(e[3] // P for e in ENCS)  # 15
        z_mm_done = 0

        s_prev = None  # running sum of fps across encoders (gpsimd)
        n_enc_done = 0
        done_encs = []
        for name in ORDER:
            ei, (_, _, K, dh) = ENC_BY_NAME[name]
            M = dh // P
            # ---- layer 1 (h-major): h.T[dh, BC] = relu(w1.T @ x.T + b1) ----
            psums = [
                psum_l1.tile([P, BC], F32, tag="mmps", name=f"l1_{name}_{m}")
                for m in range(M)
            ]
            h_sb = h_pool.tile([P, 4, BC], FP16, tag="htile")
            kdone = 0
            for kc0 in range(0, K, 4):
                kn = min(4, K - kc0)
                xt_t = xt_pool.tile([P, 4, BC], FP16, tag="xt")
                nc.sync.dma_start(
                    xt_t[:, :kn, :],
                    xt_view[:, XT_OFF[ei] + kc0 : XT_OFF[ei] + kc0 + kn, :],
                )
                w1_t = w_pool.tile([P, 4, 512], FP16, tag="w1")
                nc.sync.dma_start(
                    w1_t[:, :kn, :dh],
                    w1[name].ap()[kc0 * P : (kc0 + kn) * P, :].rearrange(
                        "(ko p) m -> p ko m", p=P
                    ),
                )
                for m in range(M):
                    for k in range(kn):
                        nc.tensor.matmul(
                            psums[m],
                            w1_t[:, k, m * P : (m + 1) * P],
                            xt_t[:, k, :],
                            start=(kdone + k == 0),
                            stop=(kdone + k == K - 1),
                        )
                kdone += kn
            for m in range(M):
                nc.scalar.activation(
                    h_sb[:, m, :], psums[m], AF.Relu, bias=b1_sb[name][:, m : m + 1]
                )
            # ---- gate partial: z += relu(h).T' @ wgp ----
            for k in range(M):
                nc.tensor.matmul(
                    z_ps,
                    wgp_sb[name][:, k, :],
                    h_sb[:, k, :],
                    start=(z_mm_done == 0),
                    stop=(z_mm_done + 1 == Z_MM_TOTAL),
                )
                z_mm_done += 1
            # ---- layer 2 (b-major): fps[b, h] per b-tile ----
            w2_t = w_pool.tile([P, 4, 512], FP16, tag="w1")
            nc.sync.dma_start(
                w2_t[:, :M, :], w2[name].ap().rearrange("(ko p) m -> p ko m", p=P)
            )
            for bt in range(NBT):
                ps = psum_l2.tile([P, H], F32, tag="l2ps", name=f"l2_{name}_{bt}")
                for k in range(M):
                    nc.tensor.matmul(
                        ps,
                        h_sb[:, k, bt * P : (bt + 1) * P],
                        w2_t[:, k, :],
                        start=(k == 0),
                        stop=False,
                    )
                # bias row via K=1 matmul: out[m,n] += 1 * b2[n]
                nc.tensor.matmul(
                    ps, ones_row[0:1, :], b2r_sb[name][0:1, :], start=False, stop=True
                )
                nc.scalar.activation(fps_bt[:, bt, ei, :], ps, AF.Copy)
                # ss_i = sum_h fps^2 via fused accumulate
                sq = sq_pool.tile([P, H], MID, tag="sq")
                nc.scalar.activation(
                    sq,
                    fps_bt[:, bt, ei, :],
                    AF.Square,
                    accum_out=stats[:, bt, 10 + ei : 11 + ei],
                )
            # ---- pair products (d via accum) + sums; msums deferred ----
            newpairs = []
            for prev in done_encs:
                pkey = (min(prev, ei), max(prev, ei))
                pr = PAIR_IDX[pkey]
                i1, i2 = pkey
                prod = scr_pool.tile([P, NBT, H], MID, tag="prod", name=f"prod{pr}")
                for bt in range(NBT):
                    nc.vector.scalar_tensor_tensor(
                        prod[:, bt, :],
                        in0=fps_bt[:, bt, i1, :],
                        scalar=0.0,
                        in1=fps_bt[:, bt, i2, :],
                        op0=ALU.add,
                        op1=ALU.mult,
                        accum_out=stats[:, bt, pr : pr + 1],
                    )
                sum_t = scr_pool.tile([P, NBT, H], MID, tag="sum", name=f"sum{pr}")
                nc.gpsimd.tensor_add(
                    sum_t, fps_bt[:, :, i1, :], fps_bt[:, :, i2, :]
                )
                newpairs.append((pr, prod, sum_t))
            # msums for this encoder's pairs (after all its prods, so the
            # d-stat STTs stay at the head of the DVE queue); the LAST
            # encoder's msums are deferred past the phase-B emission so the
            # tiny softmax ops reach the DVE queue first
            if name == ORDER[-1]:
                late_pairs = newpairs
            else:
                for pr, prod, sum_t in newpairs:
                    mask = msk_pool.tile(
                        [P, NBT, H], MID, tag="mask", name=f"mask{pr}"
                    )
                    nc.vector.tensor_scalar(
                        mask, in0=prod, scalar1=0.0, scalar2=None, op0=ALU.is_gt
                    )
                    nc.vector.tensor_mul(msum[:, pr, :, :], mask, sum_t)
            # ---- running S on gpsimd ----
            cur = fps_bt[:, :, ei, :]
            if s_prev is None:
                s_prev = cur
            else:
                if n_enc_done == len(ORDER) - 1:
                    nc.gpsimd.tensor_add(s_b, s_prev, cur)
                else:
                    s_new = msk_pool.tile(
                        [P, NBT, H], MID, tag="sacc", name=f"sacc{n_enc_done}"
                    )
                    nc.gpsimd.tensor_add(s_new, s_prev, cur)
                    s_prev = s_new
            done_encs.append(ei)
            n_enc_done += 1
            if n_enc_done == 2:
                # phase-D weights, behind the small encoders' data
                nc.sync.dma_start(
                    ew_t, enh_w.ap().rearrange("(ko p) m -> p ko m", p=P)
                )
                fw_view = fus_w.ap().rearrange("(ko p) m -> p ko m", p=P)
                nc.sync.dma_start(fw_lo, fw_view[:, 0:4, :])
                nc.sync.dma_start(fw_hi, fw_view[:, 4:8, :])

        # ================= Phase B: softmax weights (tiny b-major ops) =====
        # fpw chain first (only needs z): ez, transpose to b-major, normalize
        ez_h = cb_pool.tile([5, BC], MID, tag="ezh", name="ez_h")
        nc.scalar.activation(ez_h, z_ps, AF.Exp, bias=z0_sb[0:5, :])
        for bt in range(NBT):
            tps = psum_l1.tile([P, 8], FP16, tag="mmps", name=f"ezT{bt}")
            nc.tensor.transpose(
                tps[:, 0:5], ez_h[0:5, bt * P : (bt + 1) * P], id_sb[0:5, 0:5]
            )
            nc.scalar.activation(
                ez_bt[:, bt, :], tps[:, 0:5], AF.Copy,
                accum_out=sez_t[:, bt : bt + 1],
            )
        nc.vector.reciprocal(rsez_t, sez_t)
        for bt in range(NBT):
            nc.vector.tensor_scalar(
                fpw_t[:, bt, :], in0=ez_bt[:, bt, :],
                scalar1=rsez_t[:, bt : bt + 1], scalar2=None, op0=ALU.mult,
            )

        # pair-sim softmax: lss = ln(ss); pl_p = lss_I + lss_J
        nc.scalar.activation(lss_t, stats[:, :, 10:15], AF.Ln)
        for p10 in range(10):
            nc.vector.tensor_add(
                pl_t[:, :, p10 : p10 + 1],
                lss_t[:, :, _I[p10] : _I[p10] + 1],
                lss_t[:, :, _J[p10] : _J[p10] + 1],
            )
        nc.scalar.activation(invnn_t, pl_t, AF.Exp, scale=-0.5)
        nc.vector.tensor_mul(sims_t, stats[:, :, 0:10], invnn_t)
        nc.scalar.activation(es_t, sims_t, AF.Exp)
        for bt in range(NBT):
            nc.vector.scalar_tensor_tensor(
                e_t[:, bt, :],
                in0=stats[:, bt, 0:10],
                scalar=0.0,
                in1=es_t[:, bt, :],
                op0=ALU.is_gt,
                op1=ALU.mult,
                accum_out=den_t[:, bt : bt + 1],
            )
        # mf = 0.2*(den<=0); wq = 0.5*e/max(den,1)
        nc.vector.tensor_scalar(
            mf_t, in0=den_t, scalar1=0.0, scalar2=0.2, op0=ALU.is_le, op1=ALU.mult
        )
        nc.vector.tensor_scalar_max(den1_t, den_t, 1.0)
        nc.vector.reciprocal(rden_t, den1_t)
        nc.vector.tensor_scalar_mul(r05_t, rden_t, 0.5)
        for bt in range(NBT):
            nc.vector.tensor_scalar(
                wq_t[:, bt, :], in0=e_t[:, bt, :],
                scalar1=r05_t[:, bt : bt + 1], scalar2=None, op0=ALU.mult,
            )

        # deferred msums for the last encoder's pairs
        for pr, prod, sum_t in late_pairs:
            mask = msk_pool.tile([P, NBT, H], MID, tag="mask", name=f"mask{pr}")
            nc.vector.tensor_scalar(
                mask, in0=prod, scalar1=0.0, scalar2=None, op0=ALU.is_gt
            )
            nc.vector.tensor_mul(msum[:, pr, :, :], mask, sum_t)

        # ================= Phase C: weighted aggregation (b-major) =========
        # wsum chains (only need fpw; overlap with wq chain)
        for bt in range(NBT):
            wb = [
                cb_pool.tile([P, H], MID, tag="wb", name=f"wb{bt}_{i}")
                for i in range(2)
            ]
            nc.vector.tensor_scalar(
                wb[0], in0=fps_bt[:, bt, 0, :],
                scalar1=fpw_t[:, bt, 0:1], scalar2=None, op0=ALU.mult,
            )
            for i in range(1, 5):
                dst = wsum_b[:, bt, :] if i == 4 else wb[i % 2]
                nc.vector.affine_then_add(
                    dst, fps_bt[:, bt, i, :], wb[(i + 1) % 2],
                    scale=fpw_t[:, bt, i : i + 1], bias=0.0,
                )
            # transpose wsum b-tile to h-major right away
            for ht in range(4):
                tps = psum_l1.tile([P, P], FP16, tag="mmps", name=f"wT{bt}_{ht}")
                nc.tensor.transpose(
                    tps, wsum_b[:, bt, ht * P : (ht + 1) * P], id_sb
                )
                nc.scalar.activation(
                    wsum_h[:, ht, bt * P : (bt + 1) * P], tps, AF.Copy
                )

        # fus-lo accumulation (keeps PE warm through C); psum held open
        fus_ps = [
            psum_l2.tile([P, BC], F32, tag="l2ps", name=f"fus_{m}")
            for m in range(3)
        ]
        # only 3 bufs in pool; m=3 uses psum_z pool after z consumed
        fus_ps.append(psum_z.tile([P, BC], F32, tag="zps", name="fus_3"))
        for m in range(4):
            for k in range(4):
                nc.tensor.matmul(
                    fus_ps[m],
                    fw_lo[:, k, m * P : (m + 1) * P],
                    wsum_h[:, k, :],
                    start=(k == 0),
                    stop=False,
                )

        # common chains, one per b-tile; warm MMs keyed off chain progress
        for bt in range(NBT):
            cb = [
                cb_pool.tile([P, H], MID, tag="cb", name=f"cb{bt}_{i}")
                for i in range(2)
            ]
            nc.vector.tensor_scalar(
                cb[0], in0=s_b[:, bt, :],
                scalar1=mf_t[:, bt : bt + 1], scalar2=None, op0=ALU.mult,
            )
            for j, pr in enumerate(ORDER_PAIRS):
                dst = common_b[:, bt, :] if j == 9 else cb[(j + 1) % 2]
                nc.vector.affine_then_add(
                    dst, msum[:, pr, bt, :], cb[j % 2],
                    scale=wq_t[:, bt, pr : pr + 1], bias=0.0,
                )
                if j in (3, 7):
                    # keep-warm MM tied to chain progress
                    wps = psum_l1.tile([8, P], F32, tag="mmps", name=f"km{bt}_{j}")
                    nc.tensor.matmul(
                        wps, dst[:, 0:8], dst[:, 0:P], start=True, stop=True
                    )
            # transpose common b-tile to h-major
            for ht in range(4):
                tps = psum_l1.tile([P, P], FP16, tag="mmps", name=f"cT{bt}_{ht}")
                nc.tensor.transpose(
                    tps, common_b[:, bt, ht * P : (ht + 1) * P], id_sb
                )
                nc.scalar.activation(
                    common_h[:, ht, bt * P : (bt + 1) * P], tps, AF.Copy
                )

        # ================= Phase D: enhance + fuse (h-major) ===============
        for m in range(4):
            ps = psum_l1.tile([P, BC], F32, tag="mmps", name=f"enh_{m}")
            for k in range(4):
                nc.tensor.matmul(
                    ps,
                    ew_t[:, k, m * P : (m + 1) * P],
                    common_h[:, k, :],
                    start=(k == 0),
                    stop=(k == 3),
                )
            gate = gate_pool.tile([P, BC], MID, tag="gate")
            nc.scalar.activation(gate, ps, AF.Sigmoid, bias=enhb_sb[:, m : m + 1])
            nc.vector.tensor_mul(enh_sb[:, m, :], common_h[:, m, :], gate)

        out_view = out.ap().rearrange("(m p) n -> p m n", p=P)
        for m in range(4):
            for k in range(4):
                nc.tensor.matmul(
                    fus_ps[m],
                    fw_hi[:, k, m * P : (m + 1) * P],
                    enh_sb[:, k, :],
                    start=False,
                    stop=(k == 3),
                )
            o_sb = gate_pool.tile([P, BC], F32, tag="osb")
            nc.scalar.activation(
                o_sb, fus_ps[m], AF.Identity, bias=fusb_sb[:, m : m + 1]
            )
            nc.sync.dma_start(out_view[:, m, :], o_sb)


def prep_inputs(inputs):
    """Host-side: build the per-core in_maps from full inputs."""
    f16 = np.float16
    x = np.asarray(inputs["fp_features"], np.float32)

    def pad_rows(a, rows):
        a = np.asarray(a, np.float32)
        if a.shape[0] == rows:
            return a
        outp = np.zeros((rows, a.shape[1]), np.float32)
        outp[: a.shape[0]] = a
        return outp

    xt_full = np.zeros((XT_K * P, B), np.float32)
    offs_in = np.cumsum([0, AP_D, MA_D, MB_D, MC_D])
    for ei, (name, din, K, dh) in enumerate(ENCS):
        seg = x[:, offs_in[ei] : offs_in[ei] + din]
        xt_full[XT_OFF[ei] * P : XT_OFF[ei] * P + din, :] = np.ascontiguousarray(seg.T)
    xt_full = xt_full.astype(f16)

    wg_w = np.asarray(inputs["wg_w"], np.float32)  # [5*H, 5]
    wg_b = np.asarray(inputs["wg_b"], np.float32)  # [5]
    common_map = {}
    z0v = wg_b.reshape(5).copy()
    for ei, (name, din, K, dh) in enumerate(ENCS):
        common_map[f"w1_{name}"] = pad_rows(inputs[f"{name}_w1"], K * P).astype(f16)
        w2f = np.asarray(inputs[f"{name}_w2"], np.float32)
        common_map[f"w2_{name}"] = w2f.astype(f16)
        common_map[f"b1_{name}"] = (
            np.asarray(inputs[f"{name}_b1"], np.float32).reshape(dh // P, P).T.copy()
        )
        b2f = np.asarray(inputs[f"{name}_b2"], np.float32)
        common_map[f"b2r_{name}"] = b2f.reshape(1, H).astype(f16)
        wg_i = wg_w[ei * H : (ei + 1) * H, :]
        common_map[f"wgp_{name}"] = (w2f @ wg_i).astype(f16)
        z0v += b2f @ wg_i
    common_map["z0"] = z0v.reshape(5, 1).astype(np.float32)
    common_map["id128"] = np.eye(P, dtype=f16)
    common_map["enh_w"] = np.asarray(inputs["enh_w"], f16)
    common_map["enh_b"] = np.asarray(inputs["enh_b"], np.float32).reshape(4, P).T.copy()
    common_map["fus_w"] = np.asarray(inputs["fus_w"], f16)
    common_map["fus_b"] = np.asarray(inputs["fus_b"], np.float32).reshape(4, P).T.copy()

    in_maps = []
    for c in range(N_CORES):
        m = dict(common_map)
        m["xt"] = np.ascontiguousarray(xt_full[:, c * BC : (c + 1) * BC])
        in_maps.append(m)
    return in_maps


_NC_CACHE = None


def kernel(**inputs) -> np.ndarray:
    global _NC_CACHE
    if _NC_CACHE is None:
        _NC_CACHE = build_bass()
    nc = _NC_CACHE
    in_maps = prep_inputs(inputs)
    res = run_bass_kernel_spmd(nc, in_maps, core_ids=list(range(N_CORES)))
    outs = [res.results[c]["out"] for c in range(N_CORES)]  # each [H, BC]
    full = np.concatenate([o.T for o in outs], axis=0)  # [B, H]
    return np.ascontiguousarray(full.astype(np.float32))


# revision 11
# speedup vs baseline: 1.0032x; 1.0032x over previous
"""Trainium2 Bass kernel for nn_CommonFeatureExtractor (v3, b-major mid).

Data-parallel over 8 NeuronCores: batch dim (4096) sharded into 8 x 512,
weights replicated.

Layer-1 GEMMs run in the transposed layout (h.T [dh, b], fed by x.T);
layer-2 GEMMs flip to "b-major": lhsT = h.T[:, b-tile] so fps comes out
as [b(128-part) x h(free)] per b-tile of 128 samples.  In b-major every
per-sample scalar (pair dot d, norm ss, softmax weights wq/fpw, fallback
mf) is a [P,1] per-partition column, so:
  - d and ss fall out of fused accum_out on ops that compute the pair
    products / squares anyway (no ones-matmuls, no row DMAs);
  - softmax chains are tiny [128,10] ops; reciprocal is [128,4];
  - no partition-broadcast matmuls;
  - masked aggregation = chain of fused affine_then_add ops
    (acc = msum_p * wq_p[P,1] + acc), one DVE op per pair.

v3 vs v2: every wide elementwise op uses a FLAT 2-D [P, N] access
pattern (fps stored [P, 5, NBT*H], msum [P, 10, NBT*H]) — 3-D strided
APs were measured to fall out of the DVE 2x/4x perf modes and pay
per-run init (~2.7x slower).  The last encoder's pair dots accumulate on
the Scalar engine (prod = one wide 2x TT mul, d = per-bt ACT
Copy+accum_out) so phase B isn't stuck behind wide STTs.  wsum's
weighted sum runs muls on ACT (scale=[P,1] AP) with adds on DVE.
"""

import numpy as np

import concourse.bass as bass
import concourse.mybir as mybir
import concourse.tile as tile
from concourse import bacc
from concourse.bass_utils import run_bass_kernel_spmd

F32 = mybir.dt.float32
FP16 = mybir.dt.float16
ALU = mybir.AluOpType
AF = mybir.ActivationFunctionType

N_CORES = 8
B = 4096
BC = B // N_CORES  # 512 samples per core
H = 512
P = 128
NBT = BC // P  # 4 b-tiles per core
BH = NBT * H   # flat free size of one [all-samples, H] slab

AP_D, MA_D, MB_D, MC_D, PH_D = 2048, 167, 2048, 2048, 27
ENCS = [
    ("ap", AP_D, 16, 512),
    ("ma", MA_D, 2, 256),
    ("mb", MB_D, 16, 512),
    ("mc", MC_D, 16, 512),
    ("ph", PH_D, 1, 128),
]
XT_K = sum(e[2] for e in ENCS)  # 51 padded k-tiles of x
XT_OFF = np.cumsum([0] + [e[2] for e in ENCS])[:-1]

_I = [0, 0, 0, 0, 1, 1, 1, 2, 2, 3]
_J = [1, 2, 3, 4, 2, 3, 4, 3, 4, 4]
PAIR_IDX = {(_I[p], _J[p]): p for p in range(10)}
# compute order: small encoders first so pair work overlaps phase A
ORDER = ["ma", "ph", "ap", "mb", "mc"]
ENC_BY_NAME = {e[0]: (i, e) for i, e in enumerate(ENCS)}
# pair-completion order given ORDER (aggregation chains: late msums last)
ORDER_PAIRS = [6, 0, 3, 4, 8, 1, 2, 5, 7, 9]

MID = FP16


def build_bass():
    nc = bacc.Bacc("TRN2", target_bir_lowering=False, debug=False)

    xt = nc.dram_tensor("xt", [XT_K * P, BC], FP16, kind="ExternalInput")
    w1 = {}
    w2 = {}
    b1 = {}
    b2r = {}
    wgp = {}
    for name, _, K, dh in ENCS:
        w1[name] = nc.dram_tensor(f"w1_{name}", [K * P, dh], FP16, kind="ExternalInput")
        w2[name] = nc.dram_tensor(f"w2_{name}", [dh, H], FP16, kind="ExternalInput")
        b1[name] = nc.dram_tensor(f"b1_{name}", [P, dh // P], F32, kind="ExternalInput")
        b2r[name] = nc.dram_tensor(f"b2r_{name}", [1, H], FP16, kind="ExternalInput")
        wgp[name] = nc.dram_tensor(f"wgp_{name}", [dh, 5], FP16, kind="ExternalInput")
    z0 = nc.dram_tensor("z0", [5, 1], F32, kind="ExternalInput")
    id128 = nc.dram_tensor("id128", [P, P], FP16, kind="ExternalInput")
    enh_w = nc.dram_tensor("enh_w", [H, H], FP16, kind="ExternalInput")
    enh_b = nc.dram_tensor("enh_b", [P, 4], F32, kind="ExternalInput")
    fus_w = nc.dram_tensor("fus_w", [2 * H, H], FP16, kind="ExternalInput")
    fus_b = nc.dram_tensor("fus_b", [P, 4], F32, kind="ExternalInput")
    out = nc.dram_tensor("out", [H, BC], F32, kind="ExternalOutput")

    with tile.TileContext(nc) as tc:
        kernel_body(
            tc, xt, w1, w2, b1, b2r, wgp, z0, id128, enh_w, enh_b, fus_w, fus_b, out
        )
    nc.compile()
    return nc


def kernel_body(tc, xt, w1, w2, b1, b2r, wgp, z0, id128, enh_w, enh_b, fus_w,
                fus_b, out):
    nc = tc.nc
    import contextlib

    ctx = contextlib.ExitStack()
    with ctx:
        # -------- pools --------
        persist = ctx.enter_context(tc.tile_pool(name="persist", bufs=1))
        scr_pool = ctx.enter_context(tc.tile_pool(name="scr", bufs=4))
        msk_pool = ctx.enter_context(tc.tile_pool(name="msk", bufs=2))
        cb_pool = ctx.enter_context(tc.tile_pool(name="cb", bufs=2))
        wt_pool = ctx.enter_context(tc.tile_pool(name="wt", bufs=3))
        xt_pool = ctx.enter_context(tc.tile_pool(name="xtp", bufs=3))
        w_pool = ctx.enter_context(tc.tile_pool(name="wp", bufs=3))
        h_pool = ctx.enter_context(tc.tile_pool(name="hp", bufs=2))
        sq_pool = ctx.enter_context(tc.tile_pool(name="sqp", bufs=2))
        gate_pool = ctx.enter_context(tc.tile_pool(name="gatep", bufs=2))
        psum_l1 = ctx.enter_context(tc.tile_pool(name="psl1", bufs=4, space="PSUM"))
        psum_l2 = ctx.enter_context(tc.tile_pool(name="psl2", bufs=3, space="PSUM"))
        psum_z = ctx.enter_context(tc.tile_pool(name="psz", bufs=1, space="PSUM"))

        # -------- persistent tiles --------
        fps_bt = persist.tile([P, 5, BH], MID)       # b-major fps, flat slabs
        msum = persist.tile([P, 10, BH], MID)        # masked pair sums
        s_b = persist.tile([P, BH], MID)             # sum_i fps_i
        stats = persist.tile([P, NBT, 16], F32)      # cols 0-9 d, 10-14 ss
        pl_t = persist.tile([P, NBT, 10], MID)
        lss_t = persist.tile([P, NBT, 5], MID)
        invnn_t = persist.tile([P, NBT, 10], MID)
        sims_t = persist.tile([P, NBT, 10], MID)
        es_t = persist.tile([P, NBT, 10], MID)
        e_t = persist.tile([P, NBT, 10], MID)
        den_t = persist.tile([P, NBT], F32)
        den1_t = persist.tile([P, NBT], F32)
        rden_t = persist.tile([P, NBT], F32)
        r05_t = persist.tile([P, NBT], F32)
        wq_t = persist.tile([P, NBT, 10], F32)
        mf_t = persist.tile([P, NBT], F32)
        ez_bt = persist.tile([P, NBT, 5], MID)
        sez_t = persist.tile([P, NBT], F32)
        rsez_t = persist.tile([P, NBT], F32)
        fpw_t = persist.tile([P, NBT, 5], F32)
        common_b = persist.tile([P, BH], MID)
        wsum_b = persist.tile([P, BH], MID)
        common_h = persist.tile([P, 4, BC], MID)
        wsum_h = persist.tile([P, 4, BC], MID)
        enh_sb = persist.tile([P, 4, BC], MID)
        id_sb = persist.tile([P, P], FP16)
        ones_row = persist.tile([1, P], FP16)
        warmz = persist.tile([1, BC], MID)
        b1_sb = {}
        b2r_sb = {}
        wgp_sb = {}
        for name, _, K, dh in ENCS:
            b1_sb[name] = persist.tile([P, dh // P], F32, name=f"b1sb_{name}")
            b2r_sb[name] = persist.tile([1, H], FP16, name=f"b2r_{name}")
            wgp_sb[name] = persist.tile([P, dh // P, 5], FP16, name=f"wgp_{name}")
        z0_sb = persist.tile([5, 1], F32)
        enhb_sb = persist.tile([P, 4], F32)
        fusb_sb = persist.tile([P, 4], F32)
        ew_t = persist.tile([P, 4, 512], FP16, name="ew_t")
        fw_lo = persist.tile([P, 4, 512], FP16, name="fw_lo")
        fw_hi = persist.tile([P, 4, 512], FP16, name="fw_hi")

        nc.vector.memset(ones_row, 1.0)
        nc.vector.memset(warmz, 0.0)
        # PE warmup during the DMA preamble (K=1 matmuls, no DMA deps)
        for _wu in range(16):
            wu_ps = psum_l2.tile([P, H], F32, tag="l2ps", name=f"warm{_wu}")
            nc.tensor.matmul(wu_ps, ones_row, warmz, start=True, stop=True)
        nc.gpsimd.dma_start(id_sb, id128.ap())
        for name, _, K, dh in ENCS:
            nc.scalar.dma_start(b1_sb[name], b1[name].ap())
            nc.scalar.dma_start(b2r_sb[name], b2r[name].ap())
            nc.gpsimd.dma_start(
                wgp_sb[name], wgp[name].ap().rearrange("(ko p) m -> p ko m", p=P)
            )
        nc.gpsimd.dma_start(z0_sb, z0.ap())
        nc.scalar.dma_start(enhb_sb, enh_b.ap())
        nc.scalar.dma_start(fusb_sb, fus_b.ap())

        xt_view = xt.ap().rearrange("(ko p) n -> p ko n", p=P)

        def fps_blk(i, bt):  # [P, H] flat slice of encoder i, b-tile bt
            return fps_bt[:, i, bt * H : (bt + 1) * H]

        # ================= Phase A: encoders + pair prep ==================
        z_ps = psum_z.tile([5, BC], F32, tag="zps", name="zgate")
        Z_MM_TOTAL = sum(e[3] // P for e in ENCS)  # 15
        z_mm_done = 0

        s_prev = None
        n_enc_done = 0
        done_encs = []
        late_pairs = []
        for name in ORDER:
            ei, (_, _, K, dh) = ENC_BY_NAME[name]
            M = dh // P
            # ---- layer 1 (h-major) ----
            psums = [
                psum_l1.tile([P, BC], F32, tag="mmps", name=f"l1_{name}_{m}")
                for m in range(M)
            ]
            h_sb = h_pool.tile([P, 4, BC], FP16, tag="htile")
            kdone = 0
            for kc0 in range(0, K, 4):
                kn = min(4, K - kc0)
                xt_t = xt_pool.tile([P, 4, BC], FP16, tag="xt")
                nc.sync.dma_start(
                    xt_t[:, :kn, :],
                    xt_view[:, XT_OFF[ei] + kc0 : XT_OFF[ei] + kc0 + kn, :],
                )
                w1_t = w_pool.tile([P, 4, 512], FP16, tag="w1")
                nc.sync.dma_start(
                    w1_t[:, :kn, :dh],
                    w1[name].ap()[kc0 * P : (kc0 + kn) * P, :].rearrange(
                        "(ko p) m -> p ko m", p=P
                    ),
                )
                for m in range(M):
                    for k in range(kn):
                        nc.tensor.matmul(
                            psums[m],
                            w1_t[:, k, m * P : (m + 1) * P],
                            xt_t[:, k, :],
                            start=(kdone + k == 0),
                            stop=(kdone + k == K - 1),
                        )
                kdone += kn
            for m in range(M):
                nc.scalar.activation(
                    h_sb[:, m, :], psums[m], AF.Relu, bias=b1_sb[name][:, m : m + 1]
                )
            # ---- gate partial ----
            for k in range(M):
                nc.tensor.matmul(
                    z_ps,
                    wgp_sb[name][:, k, :],
                    h_sb[:, k, :],
                    start=(z_mm_done == 0),
                    stop=(z_mm_done + 1 == Z_MM_TOTAL),
                )
                z_mm_done += 1
            # ---- layer 2 (b-major) ----
            w2_t = w_pool.tile([P, 4, 512], FP16, tag="w1")
            nc.sync.dma_start(
                w2_t[:, :M, :], w2[name].ap().rearrange("(ko p) m -> p ko m", p=P)
            )
            for bt in range(NBT):
                ps = psum_l2.tile([P, H], F32, tag="l2ps", name=f"l2_{name}_{bt}")
                for k in range(M):
                    nc.tensor.matmul(
                        ps,
                        h_sb[:, k, bt * P : (bt + 1) * P],
                        w2_t[:, k, :],
                        start=(k == 0),
                        stop=False,
                    )
                nc.tensor.matmul(
                    ps, ones_row[0:1, :], b2r_sb[name][0:1, :], start=False, stop=True
                )
                nc.scalar.activation(fps_blk(ei, bt), ps, AF.Copy)
                sq = sq_pool.tile([P, H], MID, tag="sq")
                nc.scalar.activation(
                    sq,
                    fps_blk(ei, bt),
                    AF.Square,
                    accum_out=stats[:, bt, 10 + ei : 11 + ei],
                )
            # ---- pair products + sums ----
            newpairs = []
            for prev in done_encs:
                pkey = (min(prev, ei), max(prev, ei))
                pr = PAIR_IDX[pkey]
                i1, i2 = pkey
                prod = scr_pool.tile([P, BH], MID, tag="prod", name=f"prod{pr}")
                if name == ORDER[-1]:
                    # wide 2x mul now; d accumulates on ACT right after
                    nc.vector.tensor_mul(prod, fps_bt[:, i1, :], fps_bt[:, i2, :])
                else:
                    for bt in range(NBT):
                        nc.vector.scalar_tensor_tensor(
                            prod[:, bt * H : (bt + 1) * H],
                            in0=fps_blk(i1, bt),
                            scalar=0.0,
                            in1=fps_blk(i2, bt),
                            op0=ALU.add,
                            op1=ALU.mult,
                            accum_out=stats[:, bt, pr : pr + 1],
                        )
                sum_t = scr_pool.tile([P, BH], MID, tag="sum", name=f"sum{pr}")
                nc.gpsimd.tensor_add(sum_t, fps_bt[:, i1, :], fps_bt[:, i2, :])
                newpairs.append((pr, prod, sum_t))
            if name == ORDER[-1]:
                late_pairs = newpairs
            else:
                for pr, prod, sum_t in newpairs:
                    mask = msk_pool.tile([P, BH], MID, tag="mask", name=f"mask{pr}")
                    nc.vector.tensor_scalar(
                        mask, in0=prod, scalar1=0.0, scalar2=None, op0=ALU.is_gt
                    )
                    nc.vector.tensor_mul(msum[:, pr, :], mask, sum_t)
            # ---- running S on gpsimd ----
            cur = fps_bt[:, ei, :]
            if s_prev is None:
                s_prev = cur
            else:
                if n_enc_done == len(ORDER) - 1:
                    nc.gpsimd.tensor_add(s_b, s_prev, cur)
                else:
                    s_new = msk_pool.tile([P, BH], MID, tag="sacc",
                                          name=f"sacc{n_enc_done}")
                    nc.gpsimd.tensor_add(s_new, s_prev, cur)
                    s_prev = s_new
            done_encs.append(ei)
            n_enc_done += 1
            if n_enc_done == 2:
                nc.sync.dma_start(
                    ew_t, enh_w.ap().rearrange("(ko p) m -> p ko m", p=P)
                )
                fw_view = fus_w.ap().rearrange("(ko p) m -> p ko m", p=P)
                nc.sync.dma_start(fw_lo, fw_view[:, 0:4, :])
                nc.sync.dma_start(fw_hi, fw_view[:, 4:8, :])

        # ---- late-pair dots on ACT (bt-ascending so B chains start early) --
        for bt in range(NBT):
            for pr, prod, sum_t in late_pairs:
                dj = sq_pool.tile([P, H], MID, tag="sq", name=f"dj{pr}_{bt}")
                nc.scalar.activation(
                    dj,
                    prod[:, bt * H : (bt + 1) * H],
                    AF.Copy,
                    accum_out=stats[:, bt, pr : pr + 1],
                )

        # ================= Phase B =================
        # fpw chain (needs z only)
        ez_h = cb_pool.tile([5, BC], MID, tag="ezh", name="ez_h")
        nc.scalar.activation(ez_h, z_ps, AF.Exp, bias=z0_sb[0:5, :])
        for bt in range(NBT):
            tps = psum_l1.tile([P, 8], FP16, tag="mmps", name=f"ezT{bt}")
            nc.tensor.transpose(
                tps[:, 0:5], ez_h[0:5, bt * P : (bt + 1) * P], id_sb[0:5, 0:5]
            )
            nc.scalar.activation(
                ez_bt[:, bt, :], tps[:, 0:5], AF.Copy,
                accum_out=sez_t[:, bt : bt + 1],
            )
        nc.vector.reciprocal(rsez_t, sez_t)
        for bt in range(NBT):
            nc.vector.tensor_scalar(
                fpw_t[:, bt, :], in0=ez_bt[:, bt, :],
                scalar1=rsez_t[:, bt : bt + 1], scalar2=None, op0=ALU.mult,
            )

        # wsum chains: muls on ACT (scale = [P,1] AP), adds on DVE
        for bt in range(NBT):
            wts = []
            for i in range(5):
                wt = wt_pool.tile([P, H], MID, tag="wt", name=f"wt{bt}_{i}")
                nc.scalar.activation(
                    wt, fps_blk(i, bt), AF.Copy, scale=fpw_t[:, bt, i : i + 1]
                )
                wts.append(wt)
            u1 = cb_pool.tile([P, H], MID, tag="wu", name=f"wu1_{bt}")
            u2 = cb_pool.tile([P, H], MID, tag="wu2", name=f"wu2_{bt}")
            nc.vector.tensor_add(u1, wts[0], wts[1])
            nc.vector.tensor_add(u2, wts[2], wts[3])
            u3 = cb_pool.tile([P, H], MID, tag="wu", name=f"wu3_{bt}")
            nc.vector.tensor_add(u3, u1, u2)
            nc.vector.tensor_add(wsum_b[:, bt * H : (bt + 1) * H], u3, wts[4])
            # transpose wsum b-tile to h-major right away
            for ht in range(4):
                tps = psum_l1.tile([P, P], FP16, tag="mmps", name=f"wT{bt}_{ht}")
                nc.tensor.transpose(
                    tps, wsum_b[:, bt * H + ht * P : bt * H + (ht + 1) * P], id_sb
                )
                nc.scalar.activation(
                    wsum_h[:, ht, bt * P : (bt + 1) * P], tps, AF.Copy
                )

        # pair-sim softmax
        nc.scalar.activation(lss_t, stats[:, :, 10:15], AF.Ln)
        for p10 in range(10):
            nc.vector.tensor_add(
                pl_t[:, :, p10 : p10 + 1],
                lss_t[:, :, _I[p10] : _I[p10] + 1],
                lss_t[:, :, _J[p10] : _J[p10] + 1],
            )
        nc.scalar.activation(invnn_t, pl_t, AF.Exp, scale=-0.5)
        nc.vector.tensor_mul(sims_t, stats[:, :, 0:10], invnn_t)
        nc.scalar.activation(es_t, sims_t, AF.Exp)
        for bt in range(NBT):
            nc.vector.scalar_tensor_tensor(
                e_t[:, bt, :],
                in0=stats[:, bt, 0:10],
                scalar=0.0,
                in1=es_t[:, bt, :],
                op0=ALU.is_gt,
                op1=ALU.mult,
                accum_out=den_t[:, bt : bt + 1],
            )
        nc.vector.tensor_scalar(
            mf_t, in0=den_t, scalar1=0.0, scalar2=0.2, op0=ALU.is_le, op1=ALU.mult
        )
        nc.vector.tensor_scalar_max(den1_t, den_t, 1.0)
        nc.vector.reciprocal(rden_t, den1_t)
        nc.vector.tensor_scalar_mul(r05_t, rden_t, 0.5)
        for bt in range(NBT):
            nc.vector.tensor_scalar(
                wq_t[:, bt, :], in0=e_t[:, bt, :],
                scalar1=r05_t[:, bt : bt + 1], scalar2=None, op0=ALU.mult,
            )

        # deferred msums for the last encoder's pairs (wide flat 2x/4x ops)
        for pr, prod, sum_t in late_pairs:
            mask = msk_pool.tile([P, BH], MID, tag="mask", name=f"mask{pr}")
            nc.vector.tensor_scalar(
                mask, in0=prod, scalar1=0.0, scalar2=None, op0=ALU.is_gt
            )
            nc.vector.tensor_mul(msum[:, pr, :], mask, sum_t)

        # ================= Phase C: weighted aggregation (b-major) =========
        for bt in range(NBT):
            cb = [
                cb_pool.tile([P, H], MID, tag="cb", name=f"cb{bt}_{i}")
                for i in range(2)
            ]
            nc.vector.tensor_scalar(
                cb[0], in0=s_b[:, bt * H : (bt + 1) * H],
                scalar1=mf_t[:, bt : bt + 1], scalar2=None, op0=ALU.mult,
            )
            for j, pr in enumerate(ORDER_PAIRS):
                dst = (
                    common_b[:, bt * H : (bt + 1) * H]
                    if j == 9
                    else cb[(j + 1) % 2]
                )
                nc.vector.affine_then_add(
                    dst, msum[:, pr, bt * H : (bt + 1) * H], cb[j % 2],
                    scale=wq_t[:, bt, pr : pr + 1], bias=0.0,
                )
                if j in (2, 5, 8):
                    wps = psum_l1.tile([8, P], F32, tag="mmps", name=f"km{bt}_{j}")
                    nc.tensor.matmul(
                        wps, dst[:, 0:8], dst[:, 0:P], start=True, stop=True
                    )
            for ht in range(4):
                tps = psum_l1.tile([P, P], FP16, tag="mmps", name=f"cT{bt}_{ht}")
                nc.tensor.transpose(
                    tps, common_b[:, bt * H + ht * P : bt * H + (ht + 1) * P], id_sb
                )
                nc.scalar.activation(
                    common_h[:, ht, bt * P : (bt + 1) * P], tps, AF.Copy
                )

        # fus-lo accumulation (PE work spread through C); psums held open
        fus_ps = [
            psum_l2.tile([P, BC], F32, tag="l2ps", name=f"fus_{m}")
            for m in range(3)
        ]
        fus_ps.append(psum_z.tile([P, BC], F32, tag="zps", name="fus_3"))
        for m in range(4):
            for k in range(4):
                nc.tensor.matmul(
                    fus_ps[m],
                    fw_lo[:, k, m * P : (m + 1) * P],
                    wsum_h[:, k, :],
                    start=(k == 0),
                    stop=False,
                )

        # ================= Phase D: enhance + fuse (h-major) ===============
        for m in range(4):
            ps = psum_l1.tile([P, BC], F32, tag="mmps", name=f"enh_{m}")
            for k in range(4):
                nc.tensor.matmul(
                    ps,
                    ew_t[:, k, m * P : (m + 1) * P],
                    common_h[:, k, :],
                    start=(k == 0),
                    stop=(k == 3),
                )
            gate = gate_pool.tile([P, BC], MID, tag="gate")
            nc.scalar.activation(gate, ps, AF.Sigmoid, bias=enhb_sb[:, m : m + 1])
            nc.vector.tensor_mul(enh_sb[:, m, :], common_h[:, m, :], gate)

        out_view = out.ap().rearrange("(m p) n -> p m n", p=P)
        for m in range(4):
            for k in range(4):
                nc.tensor.matmul(
                    fus_ps[m],
                    fw_hi[:, k, m * P : (m + 1) * P],
                    enh_sb[:, k, :],
                    start=False,
                    stop=(k == 3),
                )
            o_sb = gate_pool.tile([P, BC], F32, tag="osb")
            nc.scalar.activation(
                o_sb, fus_ps[m], AF.Identity, bias=fusb_sb[:, m : m + 1]
            )
            nc.sync.dma_start(out_view[:, m, :], o_sb)


def prep_inputs(inputs):
    """Host-side: build the per-core in_maps from full inputs."""
    f16 = np.float16
    x = np.asarray(inputs["fp_features"], np.float32)

    def pad_rows(a, rows):
        a = np.asarray(a, np.float32)
        if a.shape[0] == rows:
            return a
        outp = np.zeros((rows, a.shape[1]), np.float32)
        outp[: a.shape[0]] = a
        return outp

    xt_full = np.zeros((XT_K * P, B), np.float32)
    offs_in = np.cumsum([0, AP_D, MA_D, MB_D, MC_D])
    for ei, (name, din, K, dh) in enumerate(ENCS):
        seg = x[:, offs_in[ei] : offs_in[ei] + din]
        xt_full[XT_OFF[ei] * P : XT_OFF[ei] * P + din, :] = np.ascontiguousarray(seg.T)
    xt_full = xt_full.astype(f16)

    wg_w = np.asarray(inputs["wg_w"], np.float32)
    wg_b = np.asarray(inputs["wg_b"], np.float32)
    common_map = {}
    z0v = wg_b.reshape(5).copy()
    for ei, (name, din, K, dh) in enumerate(ENCS):
        common_map[f"w1_{name}"] = pad_rows(inputs[f"{name}_w1"], K * P).astype(f16)
        w2f = np.asarray(inputs[f"{name}_w2"], np.float32)
        common_map[f"w2_{name}"] = w2f.astype(f16)
        common_map[f"b1_{name}"] = (
            np.asarray(inputs[f"{name}_b1"], np.float32).reshape(dh // P, P).T.copy()
        )
        b2f = np.asarray(inputs[f"{name}_b2"], np.float32)
        common_map[f"b2r_{name}"] = b2f.reshape(1, H).astype(f16)
        wg_i = wg_w[ei * H : (ei + 1) * H, :]
        common_map[f"wgp_{name}"] = (w2f @ wg_i).astype(f16)
        z0v += b2f @ wg_i
    common_map["z0"] = z0v.reshape(5, 1).astype(np.float32)
    common_map["id128"] = np.eye(P, dtype=f16)
    common_map["enh_w"] = np.asarray(inputs["enh_w"], f16)
    common_map["enh_b"] = np.asarray(inputs["enh_b"], np.float32).reshape(4, P).T.copy()
    common_map["fus_w"] = np.asarray(inputs["fus_w"], f16)
    common_map["fus_b"] = np.asarray(inputs["fus_b"], np.float32).reshape(4, P).T.copy()

    in_maps = []
    for c in range(N_CORES):
        m = dict(common_map)
        m["xt"] = np.ascontiguousarray(xt_full[:, c * BC : (c + 1) * BC])
        in_maps.append(m)
    return in_maps


_NC_CACHE = None


def kernel(**inputs) -> np.ndarray:
    global _NC_CACHE
    if _NC_CACHE is None:
        _NC_CACHE = build_bass()
    nc = _NC_CACHE
    in_maps = prep_inputs(inputs)
    res = run_bass_kernel_spmd(nc, in_maps, core_ids=list(range(N_CORES)))
    outs = [res.results[c]["out"] for c in range(N_CORES)]
    full = np.concatenate([o.T for o in outs], axis=0)
    return np.ascontiguousarray(full.astype(np.float32))
